# revision 1
# baseline (speedup 1.0000x reference)
"""Trainium2 Bass kernel for nn_CrossAttention_37220186587177.

Cross-attention: B=2, L=S=2048, D=1024, H=16 heads, Dh=64, RoPE on q/k,
softmax over S, out-projection.

Sharding (8 NeuronCores): data-parallel over B (2 groups of 4 cores),
tensor-parallel over heads within a group (4 heads/core).  Each core
computes its 4 heads end-to-end plus a partial out-projection over its
256 contraction dims; the 4 partial [L, D] outputs per batch are summed
on the host, and bo is added there.

v2 design notes (vs the v1 baseline at 588us):
 - q/k row layout is per-head interleaved [h0e|h0o|h1e|h1o] (32-row
   blocks), via host-side permutation of Wq/Wk output columns.  Scores
   then need ONE K=64 matmul per (head, key-block) instead of two K=32
   matmuls: 256 score matmuls instead of 512.
 - RoPE's cross-term row swap runs on the PE as a signed permutation
   matmul (P stationary), leaving only 3 full-width DVE ops per chunk.
 - Softmax denominators still come free from a ones-column appended to
   V (M=65 PV matmuls); the per-head reciprocal uses
   reciprocal_approx_fast (~5x faster than reciprocal) and the
   normalize chain is scheduled off the PE critical path.
 - Phase order K->V->Q then attention, DMAs issued in consumption
   order, head order [2,3,0,1] per l-chunk so the out-projection's
   first accumulation half never waits on the last head's normalize.
 - All matmuls in float32r (1 cycle/row at free dim >=256).
"""
import sys

if "/opt/trn_rl_repo" not in sys.path:
    sys.path.insert(0, "/opt/trn_rl_repo")

import numpy as np

import concourse.bacc as bacc
import concourse.mybir as mybir
import concourse.tile as tile
from concourse import bass_utils
from concourse.bass import ts

B, L, S, D, H, Dh = 2, 2048, 2048, 1024, 16, 64
NCORES = 8
HPC = 4              # heads per core
Dc = HPC * Dh        # 256 per-core head dims
F32 = mybir.dt.float32
F32R = mybir.dt.float32r
AF = mybir.ActivationFunctionType
SCALE = Dh ** -0.5   # 0.125


def build_nc(mm_dtype="f32r"):
    DT = {"f32r": F32R, "bf16": mybir.dt.bfloat16, "f32": F32}[mm_dtype]
    nc = bacc.Bacc("TRN2", target_bir_lowering=False, debug=False)

    xT = nc.dram_tensor("xT", [D, L], DT, kind="ExternalInput")
    eT = nc.dram_tensor("eT", [D, S], DT, kind="ExternalInput")
    wq = nc.dram_tensor("wq", [D, Dc], DT, kind="ExternalInput")
    wk = nc.dram_tensor("wk", [D, Dc], DT, kind="ExternalInput")
    wv = nc.dram_tensor("wv", [D, HPC * (Dh + 1)], DT, kind="ExternalInput")
    wo = nc.dram_tensor("wo", [Dc, D], DT, kind="ExternalInput")
    qkb = nc.dram_tensor("qkb", [128, 4], F32, kind="ExternalInput")
    smalls = nc.dram_tensor("smalls", [1, 512], DT, kind="ExternalInput")
    cost = nc.dram_tensor("cost", [128, S], DT, kind="ExternalInput")
    sint = nc.dram_tensor("sint", [128, S], DT, kind="ExternalInput")
    rper = nc.dram_tensor("rper", [128, 128], DT, kind="ExternalInput")
    vmask = nc.dram_tensor("vmask", [128, 16], F32, kind="ExternalInput")
    y = nc.dram_tensor("y", [L, D], F32, kind="ExternalOutput")

    with tile.TileContext(nc) as tc:
        with tc.tile_pool(name="const", bufs=1) as cpool, \
             tc.tile_pool(name="actin", bufs=10) as apool, \
             tc.tile_pool(name="qk", bufs=1) as qkpool, \
             tc.tile_pool(name="tmp", bufs=4) as tpool, \
             tc.tile_pool(name="vsb", bufs=1) as vpool, \
             tc.tile_pool(name="ex", bufs=4) as epool, \
             tc.tile_pool(name="on", bufs=4) as onpool, \
             tc.tile_pool(name="bc", bufs=2) as bcpool, \
             tc.tile_pool(name="rc", bufs=4) as rcpool, \
             tc.tile_pool(name="yo", bufs=3) as ypool, \
             tc.tile_pool(name="scp", bufs=3, space="PSUM") as scp, \
             tc.tile_pool(name="pvp", bufs=2, space="PSUM") as pvp, \
             tc.tile_pool(name="oup", bufs=3, space="PSUM") as oup:

            # ---- constants ----
            # weights/tables go on the Activation HWDGE queue so the SP
            # queue can stream eT/xT activations without queuing behind
            # them; both queues fill SBUF concurrently at startup.
            qkb_t = cpool.tile([128, 4], F32, name="qkb_t")
            nc.scalar.dma_start(out=qkb_t[:], in_=qkb.ap())
            sm_t = cpool.tile([1, 512], DT, name="sm_t")
            nc.scalar.dma_start(out=sm_t[:], in_=smalls.ap())
            vmask_t = cpool.tile([128, 16], F32, name="vmask_t")
            nc.scalar.dma_start(out=vmask_t[:], in_=vmask.ap())
            rper_t = cpool.tile([128, 128], DT, name="rper_t")
            nc.scalar.dma_start(out=rper_t[:], in_=rper.ap())

            w_k = cpool.tile([128, 8, Dc], DT, name="w_k")
            nc.scalar.dma_start(out=w_k[:], in_=wk.ap().rearrange("(a p) m -> p a m", p=128))
            w_v = cpool.tile([128, 8, HPC * (Dh + 1)], DT, name="w_v")
            nc.scalar.dma_start(out=w_v[:], in_=wv.ap().rearrange("(a p) m -> p a m", p=128))
            cost_t = cpool.tile([128, S], DT, name="cost_t")
            sint_t = cpool.tile([128, S], DT, name="sint_t")
            nc.scalar.dma_start(out=cost_t[:], in_=cost.ap())
            nc.scalar.dma_start(out=sint_t[:], in_=sint.ap())
            w_q = cpool.tile([128, 8, Dc], DT, name="w_q")
            nc.scalar.dma_start(out=w_q[:], in_=wq.ap().rearrange("(a p) m -> p a m", p=128))
            w_o = cpool.tile([128, 2, D], DT, name="w_o")
            nc.scalar.dma_start(out=w_o[:], in_=wo.ap().rearrange("(a p) m -> p a m", p=128))

            # persistent q/k tiles: [p][128, seq], rows = [h(2p)e|h(2p)o|h(2p+1)e|h(2p+1)o]
            qp = [qkpool.tile([128, L], DT, name=f"qp{p}", tag=f"qp{p}")
                  for p in range(2)]
            kp = [qkpool.tile([128, S], DT, name=f"kp{p}", tag=f"kp{p}")
                  for p in range(2)]

            def rope_chunk(pair, c, kind):
                """Apply RoPE in-place to pair[0..1][:, 512c:512c+512].

                new = x*cos + (P^T x)*sin, P a signed 32-row-block swap."""
                for i, t in enumerate(pair):
                    sw = scp.tile([128, 512], F32, name=f"sw_{kind}{i}_{c}",
                                  tag="sc")
                    nc.tensor.matmul(sw[:], rper_t[:], t[:, ts(c, 512)],
                                     start=True, stop=True)
                    t1 = tpool.tile([128, 512], DT, name=f"r1_{kind}{i}_{c}",
                                    tag="tmp")
                    nc.vector.tensor_mul(t1[:], t[:, ts(c, 512)],
                                         cost_t[:, ts(c, 512)])
                    t2 = tpool.tile([128, 512], DT, name=f"r2_{kind}{i}_{c}",
                                    tag="tmp")
                    nc.vector.tensor_mul(t2[:], sw[:], sint_t[:, ts(c, 512)])
                    nc.vector.tensor_add(t[:, ts(c, 512)], t1[:], t2[:])

            # ---- phase 1a: K projection + V projection + RoPE(k) ----
            v_tiles = []
            for sc in range(4):
                e_tiles = []
                for d in range(8):
                    t = apool.tile([128, 512], DT, name=f"e_{d}_{sc}", tag="act")
                    nc.sync.dma_start(out=t[:], in_=eT.ap()[ts(d, 128), ts(sc, 512)])
                    e_tiles.append(t)
                for m in range(2):
                    ps = scp.tile([128, 512], F32, name=f"kps_{m}_{sc}", tag="sc")
                    for d in range(8):
                        nc.tensor.matmul(ps[:], w_k[:, d, ts(m, 128)], e_tiles[d][:],
                                         start=(d == 0), stop=(d == 7))
                    nc.vector.tensor_scalar_add(kp[m][:, ts(sc, 512)], ps[:],
                                                qkb_t[:, m + 2:m + 3])
                for sb in range(4):
                    s_blk = 4 * sc + sb
                    wid = HPC * (Dh + 1)  # 260: per head [v(64) | ones-col]
                    ps = scp.tile([128, wid], F32, name=f"vps_{s_blk}", tag="sc")
                    # bias+ones first (start=True): v-cols get bv, 65th col 1.0
                    nc.tensor.matmul(ps[:], sm_t[0:1, 320:448], sm_t[0:1, 0:wid],
                                     start=True, stop=False)
                    for d in range(8):
                        nc.tensor.matmul(ps[:], e_tiles[d][:, ts(sb, 128)], w_v[:, d, :],
                                         start=False, stop=(d == 7))
                    vt = vpool.tile([128, wid], DT, name=f"v_{s_blk}",
                                    tag=f"v{s_blk}")
                    # mask fold: vt = ps * mask[s] (zeroes masked V rows)
                    nc.vector.tensor_scalar_mul(vt[:], ps[:],
                                                vmask_t[:, s_blk:s_blk + 1])
                    v_tiles.append(vt)
                rope_chunk(kp, sc, "k")

            # ---- phase 1b: Q projection + RoPE(q), pipelined by one chunk ----
            for lc in range(4):
                x_tiles = []
                for d in range(8):
                    t = apool.tile([128, 512], DT, name=f"x_{d}_{lc}", tag="act")
                    nc.sync.dma_start(out=t[:], in_=xT.ap()[ts(d, 128), ts(lc, 512)])
                    x_tiles.append(t)
                for m in range(2):
                    ps = scp.tile([128, 512], F32, name=f"qps_{m}_{lc}", tag="sc")
                    for d in range(8):
                        nc.tensor.matmul(ps[:], w_q[:, d, ts(m, 128)], x_tiles[d][:],
                                         start=(d == 0), stop=(d == 7))
                    nc.vector.tensor_scalar_add(qp[m][:, ts(lc, 512)], ps[:],
                                                qkb_t[:, m:m + 1])
                if lc > 0:
                    rope_chunk(qp, lc - 1, "q")
            rope_chunk(qp, 3, "q")

            # ---- phase 2: attention + out-projection per 512-wide l-chunk ----
            # head order [2,3,0,1]: the out-proj accumulates pr=1 (heads 2,3)
            # first, so its first half never waits on the last head's (h1)
            # normalize chain.
            HEAD_ORDER = (2, 3, 0, 1)
            for lc in range(4):
                on = {1: onpool.tile([128, 512], DT, name=f"on1_{lc}", tag="on"),
                      0: onpool.tile([128, 512], DT, name=f"on0_{lc}", tag="on")}
                pv = {}
                deferred = []  # normalize chains awaiting a late PE slot

                def emit_norm(h):
                    """reciprocal -> bc matmul -> cast -> on-mul for head h."""
                    rc = rcpool.tile([1, 512], DT, name=f"rc_{lc}_{h}", tag="rc")
                    with nc.allow_low_precision(reason="recip for softmax denom"):
                        nc.vector.reciprocal(rc[:], pv[h][Dh:Dh + 1, :])
                    deferred.append((h, rc))

                def flush_norm():
                    while deferred:
                        h, rc = deferred.pop(0)
                        bc_ps = oup.tile([64, 512], F32, name=f"bc_{lc}_{h}",
                                         tag="ou")
                        nc.tensor.matmul(bc_ps[:], sm_t[0:1, 320:384], rc[:],
                                         start=True, stop=True)
                        bc_sb = bcpool.tile([64, 512], DT, name=f"bcs_{lc}_{h}",
                                            tag="bc")
                        nc.vector.tensor_copy(bc_sb[:], bc_ps[:])
                        pr, half = h // 2, h % 2
                        nc.vector.tensor_mul(on[pr][64 * half:64 * (half + 1), :],
                                             pv[h][0:Dh, :], bc_sb[:])

                # software pipeline: PV lags 2 slots behind SC so the PE
                # streams scores while ACT exponentiates -- the PE never
                # waits on exp, keeping it hot (p-state).
                LAG = 2
                steps = [(h, kb) for h in HEAD_ORDER for kb in range(16)]
                pend = []  # (h, kb, ex) awaiting their PV matmul

                def emit_pv(h, kb, ex):
                    nc.tensor.matmul(pv[h][:], v_tiles[kb][:, ts(h, Dh + 1)],
                                     ex[:], start=(kb == 0), stop=(kb == 15))
                    if kb == 15:
                        emit_norm(h)

                for si, (h, kb) in enumerate(steps):
                    p, j = h // 2, h % 2
                    if kb == 0:
                        pv[h] = pvp.tile([Dh + 1, 512], F32, name=f"pv_{lc}_{h}",
                                         tag="pv")
                    sc_ps = scp.tile([128, 512], F32,
                                     name=f"sc_{lc}_{h}_{kb}", tag="sc")
                    nc.tensor.matmul(sc_ps[:],
                                     kp[p][64 * j:64 * j + 64, ts(kb, 128)],
                                     qp[p][64 * j:64 * j + 64, ts(lc, 512)],
                                     start=True, stop=True)
                    ex = epool.tile([128, 512], DT,
                                    name=f"ex_{lc}_{h}_{kb}", tag="ex")
                    nc.scalar.activation(ex[:], sc_ps[:], AF.Exp, scale=SCALE)
                    pend.append((h, kb, ex))
                    if len(pend) > LAG:
                        emit_pv(*pend.pop(0))
                    if kb == 5:
                        flush_norm()  # prev head's chain, late PE slot
                while pend:
                    emit_pv(*pend.pop(0))

                # out-projection: accumulate pr=1 (ready) then pr=0 (waits h1)
                for lm in range(4):
                    for jb in range(2):
                        yps = oup.tile([128, 512], F32, name=f"yps_{lc}_{lm}_{jb}",
                                       tag="ou")
                        nc.tensor.matmul(yps[:], on[1][:, ts(lm, 128)],
                                         w_o[:, 1, ts(jb, 512)],
                                         start=True, stop=False)
                        if lm == 0 and jb == 0:
                            flush_norm()  # h1's chain behind the first pr1 mm
                        nc.tensor.matmul(yps[:], on[0][:, ts(lm, 128)],
                                         w_o[:, 0, ts(jb, 512)],
                                         start=False, stop=True)
                        ysb = ypool.tile([128, 512], F32, name=f"ysb_{lc}_{lm}_{jb}",
                                         tag="y")
                        nc.vector.tensor_copy(ysb[:], yps[:])
                        nc.sync.dma_start(
                            out=y.ap()[512 * lc + 128 * lm:512 * lc + 128 * lm + 128,
                                       ts(jb, 512)],
                            in_=ysb[:])

    nc.compile()
    return nc


def make_in_maps(x, encoder_inputs, key_padding_mask, Wq, bq, Wk, bk, Wv, bv, Wo,
                 mm_dtype="f32r"):
    f32 = np.float32
    if mm_dtype == "bf16":
        import ml_dtypes
        mmdt = ml_dtypes.bfloat16
    else:
        mmdt = np.float32
    x = np.asarray(x, dtype=f32)
    enc = np.asarray(encoder_inputs, dtype=f32)
    mask = np.asarray(key_padding_mask)
    Wq = np.asarray(Wq, dtype=f32); bq = np.asarray(bq, dtype=f32)
    Wk = np.asarray(Wk, dtype=f32); bk = np.asarray(bk, dtype=f32)
    Wv = np.asarray(Wv, dtype=f32); bv = np.asarray(bv, dtype=f32)
    Wo = np.asarray(Wo, dtype=f32)

    inv_freq = (1.0 / (10000.0 ** (np.arange(0, Dh, 2, dtype=f32) / f32(Dh)))).astype(f32)
    ang = np.arange(S, dtype=f32)[:, None] * inv_freq[None, :]   # [S, 32]
    costab = np.tile(np.ascontiguousarray(np.cos(ang).T), (4, 1)).astype(f32)  # [128,S]
    sintab = np.tile(np.ascontiguousarray(np.sin(ang).T), (4, 1)).astype(f32)

    # signed swap permutation for RoPE: out = P^T x ->
    #   e-rows (32-blk 0 of each 64-blk): -x[o-row];  o-rows: +x[e-row]
    rper = np.zeros((128, 128), f32)
    for base in (0, 64):
        for i in range(32):
            rper[base + 32 + i, base + i] = -1.0
            rper[base + i, base + 32 + i] = 1.0

    xTb = [np.ascontiguousarray(x[b].T) for b in range(B)]
    eTb = [np.ascontiguousarray(enc[b].T) for b in range(B)]
    maskb = [np.ascontiguousarray(mask[b].astype(f32).reshape(16, 128).T)
             for b in range(B)]

    in_maps = []
    for core in range(NCORES):
        b = core // 4
        heads = [(core % 4) * HPC + i for i in range(HPC)]
        # interleaved per-head [e|o] ordering (32-blocks): h0e h0o h1e h1o ...
        eo = np.concatenate(
            [np.concatenate([64 * h + np.arange(0, 64, 2),
                             64 * h + np.arange(1, 64, 2)]) for h in heads])
        nat = np.concatenate([64 * h + np.arange(64) for h in heads])

        qkb = np.stack([bq[eo[:128]], bq[eo[128:]],
                        bk[eo[:128]], bk[eo[128:]]], axis=1)
        qkb = np.ascontiguousarray(qkb.astype(f32))
        # smalls: [0:260] = per-head [bv_h(64) | 1.0]; [320:448] = 1.0
        smalls = np.zeros((1, 512), f32)
        bv_pad = np.zeros((HPC, Dh + 1), f32)
        bv_pad[:, :Dh] = bv[nat].reshape(HPC, Dh)
        bv_pad[:, Dh] = 1.0
        smalls[0, :HPC * (Dh + 1)] = bv_pad.reshape(-1)
        smalls[0, 320:448] = 1.0
        # wv padded: per head 64 cols of Wv.T + one zero col
        wv_pad = np.zeros((D, HPC * (Dh + 1)), f32)
        wvT = Wv[nat, :].T.reshape(D, HPC, Dh)
        for h in range(HPC):
            wv_pad[:, h * (Dh + 1):h * (Dh + 1) + Dh] = wvT[:, h, :]

        in_maps.append({
            "xT": xTb[b].astype(mmdt),
            "eT": eTb[b].astype(mmdt),
            "wq": np.ascontiguousarray(Wq[eo, :].T).astype(mmdt),
            "wk": np.ascontiguousarray(Wk[eo, :].T).astype(mmdt),
            "wv": wv_pad.astype(mmdt),
            "wo": np.ascontiguousarray(Wo[:, nat].T).astype(mmdt),
            "qkb": qkb,
            "smalls": smalls.astype(mmdt),
            "cost": costab.astype(mmdt),
            "sint": sintab.astype(mmdt),
            "rper": rper.astype(mmdt),
            "vmask": maskb[b],
        })
    return in_maps


_CACHE = {}

MM_DTYPE = "bf16"


def _get_nc():
    if "nc" not in _CACHE:
        _CACHE["nc"] = build_nc(MM_DTYPE)
    return _CACHE["nc"]


def kernel(x, encoder_inputs, key_padding_mask, Wq, bq, Wk, bk, Wv, bv, Wo, bo,
           _results_hook=None):
    nc = _get_nc()
    in_maps = make_in_maps(x, encoder_inputs, key_padding_mask,
                           Wq, bq, Wk, bk, Wv, bv, Wo, mm_dtype=MM_DTYPE)
    res = bass_utils.run_bass_kernel_spmd(nc, in_maps, list(range(NCORES)))
    if _results_hook is not None:
        _results_hook(res)
    bo = np.asarray(bo, dtype=np.float32)
    out = np.empty((B, L, D), np.float32)
    for b in range(B):
        acc = res.results[4 * b]["y"].astype(np.float32).copy()
        for c in range(4 * b + 1, 4 * b + 4):
            acc += res.results[c]["y"]
        out[b] = acc + bo[None, :]
    return out



# revision 7
# speedup vs baseline: 1.8138x; 1.8138x over previous
"""Trainium2 Bass kernel for nn_CrossAttention_37220186587177.

Cross-attention: B=2, L=S=2048, D=1024, H=16 heads, Dh=64, RoPE on q/k,
softmax over S, out-projection.

Sharding (8 NeuronCores): data-parallel over B (2 groups of 4 cores),
tensor-parallel over heads within a group (4 heads/core).  Each core
computes its 4 heads end-to-end plus a partial out-projection over its
256 contraction dims; the 4 partial [L, D] outputs per batch (bf16) are
summed on the host, and bo is added there.

v3 design (vs the v2 baseline at ~422-498us):  the v2 trace showed the
PE throttled to 1.2 GHz (HAM K=4/8) for 300us of the 428us span because
the attention loop had per-step micro-stalls waiting on the scalar
engine's exp.  v3 keeps the PE densely fed and cuts PE+ACT work:

 - Wide score tiles: per (head-pair, kb) step, two row-tiled CONCURRENT
   score matmuls (head-even rows 0-63 -> tile_position (0,0), head-odd
   rows 64-127 -> (64,0)) write the two banks of one [128,1024] PSUM
   tile; ONE wide ACT exp covers both heads (amortizes the ~230-cycle
   per-instruction ACT overhead).
 - PV as col-tiled concurrent pairs: v stationary [s=128, 64] per head
   at array col-groups (0,0)/(0,64), one [128,512] PSUM accumulator per
   pair.
 - Denominators via "replicated mask" matmul pairs: stationary is 64
   identical copies of the key-padding-mask column, so the PSUM
   accumulator holds each head's softmax denominator already broadcast
   across 64 partitions.  One reciprocal_approx_fast + one DVE mul
   normalizes a whole pair; no broadcast matmul, no [1,512] reciprocals.
 - All projection work (K/V/Q chunks, RoPE, out-proj) is interleaved
   into the attention instruction stream as PE filler so the PE never
   idles long enough for HAM to re-throttle.
 - y written back in bf16 (halves the 8MB/core writeback DMA).
"""
import sys

if "/opt/trn_rl_repo" not in sys.path:
    sys.path.insert(0, "/opt/trn_rl_repo")

import numpy as np

import concourse.bacc as bacc
import concourse.mybir as mybir
import concourse.tile as tile
from concourse import bass_utils
from concourse.bass import ts

B, L, S, D, H, Dh = 2, 2048, 2048, 1024, 16, 64
NCORES = 8
HPC = 4              # heads per core
Dc = HPC * Dh        # 256 per-core head dims
F32 = mybir.dt.float32
BF16 = mybir.dt.bfloat16
AF = mybir.ActivationFunctionType
SCALE = Dh ** -0.5   # 0.125
LAG = 2              # pv/den trail the sc/exp stream by this many kb steps


def build_nc():
    DT = BF16
    nc = bacc.Bacc("TRN2", target_bir_lowering=False, debug=False)

    xT = nc.dram_tensor("xT", [D, L], DT, kind="ExternalInput")
    eT = nc.dram_tensor("eT", [D, S], DT, kind="ExternalInput")
    wq = nc.dram_tensor("wq", [D, Dc], DT, kind="ExternalInput")
    wk = nc.dram_tensor("wk", [D, Dc], DT, kind="ExternalInput")
    wv = nc.dram_tensor("wv", [D, Dc], DT, kind="ExternalInput")
    wo = nc.dram_tensor("wo", [Dc, D], DT, kind="ExternalInput")
    qkb = nc.dram_tensor("qkb", [128, 4], F32, kind="ExternalInput")
    smalls = nc.dram_tensor("smalls", [1, 512], DT, kind="ExternalInput")
    cost = nc.dram_tensor("cost", [128, S], DT, kind="ExternalInput")
    sint = nc.dram_tensor("sint", [128, S], DT, kind="ExternalInput")
    rper = nc.dram_tensor("rper", [128, 128], DT, kind="ExternalInput")
    vmaskr = nc.dram_tensor("vmaskr", [128, 16 * 64], DT, kind="ExternalInput")
    vmask = nc.dram_tensor("vmask", [128, 16], F32, kind="ExternalInput")
    y = nc.dram_tensor("y", [L, D], DT, kind="ExternalOutput")

    with tile.TileContext(nc) as tc:
        with tc.tile_pool(name="const", bufs=1) as cpool, \
             tc.tile_pool(name="actin", bufs=26) as apool, \
             tc.tile_pool(name="qk", bufs=1) as qkpool, \
             tc.tile_pool(name="tmp", bufs=4) as tpool, \
             tc.tile_pool(name="vsb", bufs=1) as vpool, \
             tc.tile_pool(name="ex", bufs=4) as epool, \
             tc.tile_pool(name="on", bufs=4) as onpool, \
             tc.tile_pool(name="rc", bufs=2) as rcpool, \
             tc.tile_pool(name="yo", bufs=3) as ypool, \
             tc.tile_pool(name="scw", bufs=2, space="PSUM") as scwp, \
             tc.tile_pool(name="pvp", bufs=1, space="PSUM") as pvp, \
             tc.tile_pool(name="dnp", bufs=1, space="PSUM") as dnp, \
             tc.tile_pool(name="aux", bufs=2, space="PSUM") as auxp:

            # ---- constants ----
            # weights/tables on the Activation HWDGE queue; activations
            # stream on the SP queue concurrently.
            qkb_t = cpool.tile([128, 4], F32, name="qkb_t")
            nc.scalar.dma_start(out=qkb_t[:], in_=qkb.ap())
            sm_t = cpool.tile([1, 512], DT, name="sm_t")
            nc.scalar.dma_start(out=sm_t[:], in_=smalls.ap())
            vmask_t = cpool.tile([128, 16], F32, name="vmask_t")
            nc.scalar.dma_start(out=vmask_t[:], in_=vmask.ap())
            vmr_t = cpool.tile([128, 16, 64], DT, name="vmr_t")
            nc.scalar.dma_start(out=vmr_t[:], in_=vmaskr.ap().rearrange(
                "p (k m) -> p k m", k=16))
            rper_t = cpool.tile([128, 128], DT, name="rper_t")
            nc.scalar.dma_start(out=rper_t[:], in_=rper.ap())

            w_k = cpool.tile([128, 8, Dc], DT, name="w_k")
            nc.scalar.dma_start(out=w_k[:], in_=wk.ap().rearrange("(a p) m -> p a m", p=128))
            w_v = cpool.tile([128, 8, Dc], DT, name="w_v")
            nc.scalar.dma_start(out=w_v[:], in_=wv.ap().rearrange("(a p) m -> p a m", p=128))
            cost_t = cpool.tile([128, S], DT, name="cost_t")
            sint_t = cpool.tile([128, S], DT, name="sint_t")
            nc.scalar.dma_start(out=cost_t[:], in_=cost.ap())
            nc.scalar.dma_start(out=sint_t[:], in_=sint.ap())
            w_q = cpool.tile([128, 8, Dc], DT, name="w_q")
            nc.scalar.dma_start(out=w_q[:], in_=wq.ap().rearrange("(a p) m -> p a m", p=128))
            w_o = cpool.tile([128, 2, D], DT, name="w_o")
            nc.scalar.dma_start(out=w_o[:], in_=wo.ap().rearrange("(a p) m -> p a m", p=128))

            # persistent q/k tiles: [p][128, seq], rows = [h(2p)e|h(2p)o|h(2p+1)e|h(2p+1)o]
            qp = [qkpool.tile([128, L], DT, name=f"qp{p}", tag=f"qp{p}")
                  for p in range(2)]
            kp = [qkpool.tile([128, S], DT, name=f"kp{p}", tag=f"kp{p}")
                  for p in range(2)]
            v_tiles = [vpool.tile([128, Dc], DT, name=f"v_{s_blk}",
                                  tag=f"v{s_blk}") for s_blk in range(16)]

            def rope_chunk_ops(pair, c, kind):
                """Yield single-op closures applying RoPE in-place to
                pair[0..1][:, 512c:512c+512]."""
                for i in range(2):
                    t = pair[i]

                    def mk(i=i, t=t):
                        sw = auxp.tile([128, 512], F32,
                                       name=f"sw_{kind}{i}_{c}", tag="aux")
                        nc.tensor.matmul(sw[:], rper_t[:], t[:, ts(c, 512)],
                                         start=True, stop=True)
                        t1 = tpool.tile([128, 512], DT,
                                        name=f"r1_{kind}{i}_{c}", tag="tmp")
                        nc.vector.tensor_mul(t1[:], t[:, ts(c, 512)],
                                             cost_t[:, ts(c, 512)])
                        t2 = tpool.tile([128, 512], DT,
                                        name=f"r2_{kind}{i}_{c}", tag="tmp")
                        nc.vector.tensor_mul(t2[:], sw[:], sint_t[:, ts(c, 512)])
                        nc.vector.tensor_add(t[:, ts(c, 512)], t1[:], t2[:])
                    yield mk

            def kproj_chunk_ops(sc):
                """K projection for key chunk sc (512 keys): closures."""
                e_tiles = []

                def dma(sc=sc):
                    for d in range(8):
                        t = apool.tile([128, 512], DT, name=f"e_{d}_{sc}",
                                       tag="act")
                        nc.sync.dma_start(out=t[:],
                                          in_=eT.ap()[ts(d, 128), ts(sc, 512)])
                        e_tiles.append(t)
                yield dma
                for m in range(2):
                    def mk(m=m, sc=sc):
                        ps = auxp.tile([128, 512], F32, name=f"kps_{m}_{sc}",
                                       tag="aux")
                        for d in range(8):
                            nc.tensor.matmul(ps[:], w_k[:, d, ts(m, 128)],
                                             e_tiles[d][:],
                                             start=(d == 0), stop=(d == 7))
                        nc.vector.tensor_scalar_add(kp[m][:, ts(sc, 512)],
                                                    ps[:],
                                                    qkb_t[:, m + 2:m + 3])
                    yield mk
                for sb in range(4):
                    def mk(sb=sb, sc=sc):
                        s_blk = 4 * sc + sb
                        ps = auxp.tile([128, Dc], F32, name=f"vps_{s_blk}",
                                       tag="aux")
                        # bias first (start=True): every row gets bv
                        nc.tensor.matmul(ps[:], sm_t[0:1, 320:448],
                                         sm_t[0:1, 0:Dc],
                                         start=True, stop=False)
                        for d in range(8):
                            nc.tensor.matmul(ps[:], e_tiles[d][:, ts(sb, 128)],
                                             w_v[:, d, :],
                                             start=False, stop=(d == 7))
                        # mask fold: zero out masked key rows of V
                        nc.vector.tensor_scalar_mul(v_tiles[s_blk][:], ps[:],
                                                    vmask_t[:, s_blk:s_blk + 1])
                    yield mk
                yield from rope_chunk_ops(kp, sc, "k")

            def qproj_chunk_ops(lc):
                x_tiles = []

                def dma(lc=lc):
                    for d in range(8):
                        t = apool.tile([128, 512], DT, name=f"x_{d}_{lc}",
                                       tag="act")
                        nc.sync.dma_start(out=t[:],
                                          in_=xT.ap()[ts(d, 128), ts(lc, 512)])
                        x_tiles.append(t)
                yield dma
                for m in range(2):
                    def mk(m=m, lc=lc):
                        ps = auxp.tile([128, 512], F32, name=f"qps_{m}_{lc}",
                                       tag="aux")
                        for d in range(8):
                            nc.tensor.matmul(ps[:], w_q[:, d, ts(m, 128)],
                                             x_tiles[d][:],
                                             start=(d == 0), stop=(d == 7))
                        nc.vector.tensor_scalar_add(qp[m][:, ts(lc, 512)],
                                                    ps[:], qkb_t[:, m:m + 1])
                    yield mk
                yield from rope_chunk_ops(qp, lc, "q")

            def outproj_ops(lc, on):
                """Out-projection for l-chunk lc from on[pair] tiles."""
                for jb in range(2):
                    for lm in range(4):
                        def mk(jb=jb, lm=lm, lc=lc, on=on):
                            yps = auxp.tile([128, 512], F32,
                                            name=f"yps_{lc}_{lm}_{jb}",
                                            tag="aux")
                            nc.tensor.matmul(yps[:], on[1][:, ts(lm, 128)],
                                             w_o[:, 1, ts(jb, 512)],
                                             start=True, stop=False)
                            nc.tensor.matmul(yps[:], on[0][:, ts(lm, 128)],
                                             w_o[:, 0, ts(jb, 512)],
                                             start=False, stop=True)
                            ysb = ypool.tile([128, 512], DT,
                                             name=f"ysb_{lc}_{lm}_{jb}",
                                             tag="y")
                            nc.vector.tensor_copy(ysb[:], yps[:])
                            nc.sync.dma_start(
                                out=y.ap()[512 * lc + 128 * lm:
                                           512 * lc + 128 * lm + 128,
                                           ts(jb, 512)],
                                in_=ysb[:])
                        yield mk

            # ================= instruction stream =================
            # Prefix: K/V chunk 0, Q chunk 0 (with RoPE), so lc0/kb0-3
            # attention can start; everything else interleaves as filler.
            for op in kproj_chunk_ops(0):
                op()
            for op in qproj_chunk_ops(0):
                op()

            def attention_lc(lc, fillers, per_step):
                """Attention for l-chunk lc.  `fillers` is an iterator of
                closures; `per_step` of them are drained after each kb
                step (the remainder drains at pair boundaries as the
                dependency order allows)."""
                on = {}

                def drain(n):
                    for _ in range(n):
                        op = next(fillers, None)
                        if op is None:
                            return
                        op()

                for pr in (1, 0):
                    pvt = pvp.tile([128, 512], F32, name=f"pv_{lc}_{pr}",
                                   tag="pv")
                    dnt = dnp.tile([128, 512], F32, name=f"dn_{lc}_{pr}",
                                   tag="dn")
                    pend = []

                    def emit_pv(kb, ex_w, pvt=pvt, dnt=dnt, pr=pr):
                        vt = v_tiles[kb]
                        nc.tensor.matmul(pvt[0:64, :],
                                         vt[:, 128 * pr:128 * pr + 64],
                                         ex_w[:, 0:512],
                                         start=(kb == 0), stop=(kb == 15))
                        nc.tensor.matmul(pvt[64:128, :],
                                         vt[:, 128 * pr + 64:128 * pr + 128],
                                         ex_w[:, 512:1024],
                                         start=(kb == 0), stop=(kb == 15))
                        nc.tensor.matmul(dnt[0:64, :], vmr_t[:, kb, :],
                                         ex_w[:, 0:512],
                                         start=(kb == 0), stop=(kb == 15))
                        nc.tensor.matmul(dnt[64:128, :], vmr_t[:, kb, :],
                                         ex_w[:, 512:1024],
                                         start=(kb == 0), stop=(kb == 15))

                    for kb in range(16):
                        drain(per_step)
                        wide = scwp.tile([128, 1024], F32,
                                         name=f"sc_{lc}_{pr}_{kb}", tag="sc")
                        nc.tensor.matmul(wide[:, 0:512],
                                         kp[pr][0:64, ts(kb, 128)],
                                         qp[pr][0:64, ts(lc, 512)],
                                         start=True, stop=True)
                        nc.tensor.matmul(wide[:, 512:1024],
                                         kp[pr][64:128, ts(kb, 128)],
                                         qp[pr][64:128, ts(lc, 512)],
                                         start=True, stop=True)
                        ex_w = epool.tile([128, 1024], DT,
                                          name=f"ex_{lc}_{pr}_{kb}", tag="ex")
                        nc.scalar.activation(ex_w[:], wide[:], AF.Exp,
                                             scale=SCALE)
                        pend.append((kb, ex_w))
                        if len(pend) > LAG:
                            emit_pv(*pend.pop(0))
                    while pend:
                        emit_pv(*pend.pop(0))

                    # normalize: rc = 1/denoms (already broadcast across
                    # partitions), on = pv * rc
                    rc = rcpool.tile([128, 512], F32, name=f"rc_{lc}_{pr}",
                                     tag="rc")
                    nc.vector.reciprocal_approx_fast(out=rc[:], in_=dnt[:])
                    on[pr] = onpool.tile([128, 512], DT, name=f"on_{lc}_{pr}",
                                         tag="on")
                    nc.vector.tensor_mul(on[pr][:], pvt[:], rc[:])
                return on

            # filler schedules per l-chunk
            def chain(*gens):
                for g in gens:
                    yield from g

            def eager_dma(gen):
                """Run the generator's leading DMA closure now (prefetch);
                yield the remaining compute closures lazily."""
                next(gen)()
                return gen

            # prefetch the remaining eT chunks and the lc1 xT chunk now so
            # the lc0 filler windows line up (chunk c complete by kb=4c):
            # exactly 8 compute closures per K chunk at 2 drains/step.
            lc0_fillers = chain(eager_dma(kproj_chunk_ops(1)),
                                eager_dma(kproj_chunk_ops(2)),
                                eager_dma(kproj_chunk_ops(3)),
                                eager_dma(qproj_chunk_ops(1)))
            on0 = attention_lc(0, lc0_fillers, per_step=2)
            on1 = attention_lc(1, chain(outproj_ops(0, on0),
                                        qproj_chunk_ops(2)), per_step=1)
            on2 = attention_lc(2, chain(outproj_ops(1, on1),
                                        qproj_chunk_ops(3)), per_step=1)
            on3 = attention_lc(3, outproj_ops(2, on2), per_step=1)
            for op in outproj_ops(3, on3):
                op()

    nc.compile()
    return nc


def make_in_maps(x, encoder_inputs, key_padding_mask, Wq, bq, Wk, bk, Wv, bv, Wo):
    f32 = np.float32
    import ml_dtypes
    mmdt = ml_dtypes.bfloat16
    x = np.asarray(x, dtype=f32)
    enc = np.asarray(encoder_inputs, dtype=f32)
    mask = np.asarray(key_padding_mask)
    Wq = np.asarray(Wq, dtype=f32); bq = np.asarray(bq, dtype=f32)
    Wk = np.asarray(Wk, dtype=f32); bk = np.asarray(bk, dtype=f32)
    Wv = np.asarray(Wv, dtype=f32); bv = np.asarray(bv, dtype=f32)
    Wo = np.asarray(Wo, dtype=f32)

    inv_freq = (1.0 / (10000.0 ** (np.arange(0, Dh, 2, dtype=f32) / f32(Dh)))).astype(f32)
    ang = np.arange(S, dtype=f32)[:, None] * inv_freq[None, :]   # [S, 32]
    costab = np.tile(np.ascontiguousarray(np.cos(ang).T), (4, 1)).astype(f32)  # [128,S]
    sintab = np.tile(np.ascontiguousarray(np.sin(ang).T), (4, 1)).astype(f32)

    # signed swap permutation for RoPE: out = P^T x ->
    #   e-rows (32-blk 0 of each 64-blk): -x[o-row];  o-rows: +x[e-row]
    rper = np.zeros((128, 128), f32)
    for base in (0, 64):
        for i in range(32):
            rper[base + 32 + i, base + i] = -1.0
            rper[base + i, base + 32 + i] = 1.0

    xTb = [np.ascontiguousarray(x[b].T) for b in range(B)]
    eTb = [np.ascontiguousarray(enc[b].T) for b in range(B)]
    maskb = [np.ascontiguousarray(mask[b].astype(f32).reshape(16, 128).T)
             for b in range(B)]

    in_maps = []
    for core in range(NCORES):
        b = core // 4
        heads = [(core % 4) * HPC + i for i in range(HPC)]
        # interleaved per-head [e|o] ordering (32-blocks): h0e h0o h1e h1o ...
        eo = np.concatenate(
            [np.concatenate([64 * h + np.arange(0, 64, 2),
                             64 * h + np.arange(1, 64, 2)]) for h in heads])
        nat = np.concatenate([64 * h + np.arange(64) for h in heads])

        qkb = np.stack([bq[eo[:128]], bq[eo[128:]],
                        bk[eo[:128]], bk[eo[128:]]], axis=1)
        qkb = np.ascontiguousarray(qkb.astype(f32))
        # smalls: [0:256] = bv (natural head order); [320:448] = 1.0
        smalls = np.zeros((1, 512), f32)
        smalls[0, :Dc] = bv[nat]
        smalls[0, 320:448] = 1.0
        # replicated mask for denominator matmuls: [128, 16*64]
        vmaskr = np.repeat(maskb[b][:, :, None], 64, axis=2).reshape(128, 16 * 64)

        in_maps.append({
            "xT": xTb[b].astype(mmdt),
            "eT": eTb[b].astype(mmdt),
            "wq": np.ascontiguousarray(Wq[eo, :].T).astype(mmdt),
            "wk": np.ascontiguousarray(Wk[eo, :].T).astype(mmdt),
            "wv": np.ascontiguousarray(Wv[nat, :].T).astype(mmdt),
            "wo": np.ascontiguousarray(Wo[:, nat].T).astype(mmdt),
            "qkb": qkb,
            "smalls": smalls.astype(mmdt),
            "cost": costab.astype(mmdt),
            "sint": sintab.astype(mmdt),
            "rper": rper.astype(mmdt),
            "vmaskr": np.ascontiguousarray(vmaskr).astype(mmdt),
            "vmask": maskb[b],
        })
    return in_maps


_CACHE = {}


def _get_nc():
    if "nc" not in _CACHE:
        _CACHE["nc"] = build_nc()
    return _CACHE["nc"]


def kernel(x, encoder_inputs, key_padding_mask, Wq, bq, Wk, bk, Wv, bv, Wo, bo,
           _results_hook=None):
    nc = _get_nc()
    in_maps = make_in_maps(x, encoder_inputs, key_padding_mask,
                           Wq, bq, Wk, bk, Wv, bv, Wo)
    res = bass_utils.run_bass_kernel_spmd(nc, in_maps, list(range(NCORES)))
    if _results_hook is not None:
        _results_hook(res)
    bo = np.asarray(bo, dtype=np.float32)
    out = np.empty((B, L, D), np.float32)
    for b in range(B):
        acc = res.results[4 * b]["y"].astype(np.float32)
        for c in range(4 * b + 1, 4 * b + 4):
            acc = acc + res.results[c]["y"].astype(np.float32)
        out[b] = acc + bo[None, :]
    return out


# revision 14
# speedup vs baseline: 1.8148x; 1.0006x over previous
"""Trainium2 Bass kernel for nn_CrossAttention_37220186587177.

Cross-attention: B=2, L=S=2048, D=1024, H=16 heads, Dh=64, RoPE on q/k,
softmax over S, out-projection.

Sharding (8 NeuronCores): data-parallel over B (2 groups of 4 cores),
tensor-parallel over heads within a group (4 heads/core).  Each core
computes its 4 heads end-to-end plus a partial out-projection over its
256 contraction dims; the 4 partial [L, D] outputs per batch (bf16) are
summed on the host, and bo is added there.

v3 design (vs the v2 baseline at ~422-498us):  the v2 trace showed the
PE throttled to 1.2 GHz (HAM K=4/8) for 300us of the 428us span because
the attention loop had per-step micro-stalls waiting on the scalar
engine's exp.  v3 keeps the PE densely fed and cuts PE+ACT work:

 - Wide score tiles: per (head-pair, kb) step, two row-tiled CONCURRENT
   score matmuls (head-even rows 0-63 -> tile_position (0,0), head-odd
   rows 64-127 -> (64,0)) write the two banks of one [128,1024] PSUM
   tile; ONE wide ACT exp covers both heads (amortizes the ~230-cycle
   per-instruction ACT overhead).
 - PV as col-tiled concurrent pairs: v stationary [s=128, 64] per head
   at array col-groups (0,0)/(0,64), one [128,512] PSUM accumulator per
   pair.
 - Denominators via "replicated mask" matmul pairs: stationary is 64
   identical copies of the key-padding-mask column, so the PSUM
   accumulator holds each head's softmax denominator already broadcast
   across 64 partitions.  One reciprocal_approx_fast + one DVE mul
   normalizes a whole pair; no broadcast matmul, no [1,512] reciprocals.
 - All projection work (K/V/Q chunks, RoPE, out-proj) is interleaved
   into the attention instruction stream as PE filler so the PE never
   idles long enough for HAM to re-throttle.
 - y written back in bf16 (halves the 8MB/core writeback DMA).
"""
import sys

if "/opt/trn_rl_repo" not in sys.path:
    sys.path.insert(0, "/opt/trn_rl_repo")

import numpy as np

import concourse.bacc as bacc
import concourse.mybir as mybir
import concourse.tile as tile
from concourse import bass_utils
from concourse.bass import ts

B, L, S, D, H, Dh = 2, 2048, 2048, 1024, 16, 64
NCORES = 8
HPC = 4              # heads per core
Dc = HPC * Dh        # 256 per-core head dims
F32 = mybir.dt.float32
BF16 = mybir.dt.bfloat16
AF = mybir.ActivationFunctionType
SCALE = Dh ** -0.5   # 0.125
LAG = 2              # pv/den trail the sc/exp stream by this many kb steps


def build_nc():
    DT = BF16
    nc = bacc.Bacc("TRN2", target_bir_lowering=False, debug=False)

    xT = nc.dram_tensor("xT", [D, L], DT, kind="ExternalInput")
    eT = nc.dram_tensor("eT", [D, S], DT, kind="ExternalInput")
    wq = nc.dram_tensor("wq", [D, Dc], DT, kind="ExternalInput")
    wk = nc.dram_tensor("wk", [D, Dc], DT, kind="ExternalInput")
    wv = nc.dram_tensor("wv", [D, Dc], DT, kind="ExternalInput")
    wo = nc.dram_tensor("wo", [Dc, D], DT, kind="ExternalInput")
    qkb = nc.dram_tensor("qkb", [128, 4], F32, kind="ExternalInput")
    smalls = nc.dram_tensor("smalls", [1, 512], DT, kind="ExternalInput")
    cost = nc.dram_tensor("cost", [128, S], DT, kind="ExternalInput")
    sint = nc.dram_tensor("sint", [128, S], DT, kind="ExternalInput")
    rper = nc.dram_tensor("rper", [128, 128], DT, kind="ExternalInput")
    vmaskr = nc.dram_tensor("vmaskr", [128, 16 * 64], DT, kind="ExternalInput")
    vmask = nc.dram_tensor("vmask", [128, 16], F32, kind="ExternalInput")
    y = nc.dram_tensor("y", [L, D], DT, kind="ExternalOutput")

    with tile.TileContext(nc) as tc:
        with tc.tile_pool(name="const", bufs=1) as cpool, \
             tc.tile_pool(name="actin", bufs=26) as apool, \
             tc.tile_pool(name="qk", bufs=1) as qkpool, \
             tc.tile_pool(name="tmp", bufs=4) as tpool, \
             tc.tile_pool(name="vsb", bufs=1) as vpool, \
             tc.tile_pool(name="ex", bufs=4) as epool, \
             tc.tile_pool(name="on", bufs=4) as onpool, \
             tc.tile_pool(name="rc", bufs=2) as rcpool, \
             tc.tile_pool(name="yo", bufs=3) as ypool, \
             tc.tile_pool(name="scw", bufs=2, space="PSUM") as scwp, \
             tc.tile_pool(name="pvp", bufs=1, space="PSUM") as pvp, \
             tc.tile_pool(name="dnp", bufs=1, space="PSUM") as dnp, \
             tc.tile_pool(name="aux", bufs=2, space="PSUM") as auxp:

            # ---- constants ----
            # weights/tables on the Activation HWDGE queue; activations
            # stream on the SP queue concurrently.  Queue order follows the
            # consumption order so the K-projection critical path (qkb, w_k)
            # lands first and the first matmul issues ~5us earlier.
            qkb_t = cpool.tile([128, 4], F32, name="qkb_t")
            nc.scalar.dma_start(out=qkb_t[:], in_=qkb.ap())
            w_k = cpool.tile([128, 8, Dc], DT, name="w_k")
            nc.scalar.dma_start(out=w_k[:], in_=wk.ap().rearrange("(a p) m -> p a m", p=128))
            sm_t = cpool.tile([1, 512], DT, name="sm_t")
            nc.scalar.dma_start(out=sm_t[:], in_=smalls.ap())
            vmask_t = cpool.tile([128, 16], F32, name="vmask_t")
            nc.scalar.dma_start(out=vmask_t[:], in_=vmask.ap())
            w_v = cpool.tile([128, 8, Dc], DT, name="w_v")
            nc.scalar.dma_start(out=w_v[:], in_=wv.ap().rearrange("(a p) m -> p a m", p=128))
            cost_t = cpool.tile([128, S], DT, name="cost_t")
            sint_t = cpool.tile([128, S], DT, name="sint_t")
            nc.scalar.dma_start(out=cost_t[:], in_=cost.ap())
            nc.scalar.dma_start(out=sint_t[:], in_=sint.ap())
            rper_t = cpool.tile([128, 128], DT, name="rper_t")
            nc.scalar.dma_start(out=rper_t[:], in_=rper.ap())
            w_q = cpool.tile([128, 8, Dc], DT, name="w_q")
            nc.scalar.dma_start(out=w_q[:], in_=wq.ap().rearrange("(a p) m -> p a m", p=128))
            vmr_t = cpool.tile([128, 16, 64], DT, name="vmr_t")
            nc.scalar.dma_start(out=vmr_t[:], in_=vmaskr.ap().rearrange(
                "p (k m) -> p k m", k=16))
            w_o = cpool.tile([128, 2, D], DT, name="w_o")
            nc.scalar.dma_start(out=w_o[:], in_=wo.ap().rearrange("(a p) m -> p a m", p=128))

            # persistent q/k tiles: [p][128, seq], rows = [h(2p)e|h(2p)o|h(2p+1)e|h(2p+1)o]
            qp = [qkpool.tile([128, L], DT, name=f"qp{p}", tag=f"qp{p}")
                  for p in range(2)]
            kp = [qkpool.tile([128, S], DT, name=f"kp{p}", tag=f"kp{p}")
                  for p in range(2)]
            v_tiles = [vpool.tile([128, Dc], DT, name=f"v_{s_blk}",
                                  tag=f"v{s_blk}") for s_blk in range(16)]

            def rope_chunk_ops(pair, c, kind):
                """Yield single-op closures applying RoPE in-place to
                pair[0..1][:, 512c:512c+512]."""
                for i in range(2):
                    t = pair[i]

                    def mk(i=i, t=t):
                        sw = auxp.tile([128, 512], F32,
                                       name=f"sw_{kind}{i}_{c}", tag="aux")
                        nc.tensor.matmul(sw[:], rper_t[:], t[:, ts(c, 512)],
                                         start=True, stop=True)
                        t1 = tpool.tile([128, 512], DT,
                                        name=f"r1_{kind}{i}_{c}", tag="tmp")
                        nc.vector.tensor_mul(t1[:], t[:, ts(c, 512)],
                                             cost_t[:, ts(c, 512)])
                        t2 = tpool.tile([128, 512], DT,
                                        name=f"r2_{kind}{i}_{c}", tag="tmp")
                        nc.vector.tensor_mul(t2[:], sw[:], sint_t[:, ts(c, 512)])
                        nc.vector.tensor_add(t[:, ts(c, 512)], t1[:], t2[:])
                    yield mk

            def kproj_chunk_ops(sc):
                """K projection for key chunk sc (512 keys): closures."""
                e_tiles = []

                def dma(sc=sc):
                    for d in range(8):
                        t = apool.tile([128, 512], DT, name=f"e_{d}_{sc}",
                                       tag="act")
                        nc.sync.dma_start(out=t[:],
                                          in_=eT.ap()[ts(d, 128), ts(sc, 512)])
                        e_tiles.append(t)
                yield dma
                for m in range(2):
                    def mk(m=m, sc=sc):
                        ps = auxp.tile([128, 512], F32, name=f"kps_{m}_{sc}",
                                       tag="aux")
                        for d in range(8):
                            nc.tensor.matmul(ps[:], w_k[:, d, ts(m, 128)],
                                             e_tiles[d][:],
                                             start=(d == 0), stop=(d == 7))
                        nc.vector.tensor_scalar_add(kp[m][:, ts(sc, 512)],
                                                    ps[:],
                                                    qkb_t[:, m + 2:m + 3])
                    yield mk
                for sb in range(4):
                    def mk(sb=sb, sc=sc):
                        s_blk = 4 * sc + sb
                        ps = auxp.tile([128, Dc], F32, name=f"vps_{s_blk}",
                                       tag="aux")
                        # bias first (start=True): every row gets bv
                        nc.tensor.matmul(ps[:], sm_t[0:1, 320:448],
                                         sm_t[0:1, 0:Dc],
                                         start=True, stop=False)
                        for d in range(8):
                            nc.tensor.matmul(ps[:], e_tiles[d][:, ts(sb, 128)],
                                             w_v[:, d, :],
                                             start=False, stop=(d == 7))
                        # mask fold: zero out masked key rows of V
                        nc.vector.tensor_scalar_mul(v_tiles[s_blk][:], ps[:],
                                                    vmask_t[:, s_blk:s_blk + 1])
                    yield mk
                yield from rope_chunk_ops(kp, sc, "k")

            def qproj_chunk_ops(lc, fine=False):
                x_tiles = []

                def dma(lc=lc):
                    for d in range(8):
                        t = apool.tile([128, 512], DT, name=f"x_{d}_{lc}",
                                       tag="act")
                        nc.sync.dma_start(out=t[:],
                                          in_=xT.ap()[ts(d, 128), ts(lc, 512)])
                        x_tiles.append(t)
                yield dma
                for m in range(2):
                    ps_cell = []

                    def mk_mm(m=m, lc=lc, d=0, ps_cell=ps_cell):
                        if d == 0:
                            ps_cell.append(auxp.tile([128, 512], F32,
                                                     name=f"qps_{m}_{lc}",
                                                     tag="aux"))
                        nc.tensor.matmul(ps_cell[0][:], w_q[:, d, ts(m, 128)],
                                         x_tiles[d][:],
                                         start=(d == 0), stop=(d == 7))
                        if d == 7:
                            nc.vector.tensor_scalar_add(qp[m][:, ts(lc, 512)],
                                                        ps_cell[0][:],
                                                        qkb_t[:, m:m + 1])
                    if fine:
                        # one matmul per filler slot so a drain never blocks
                        # the attention stream for more than ~220ns
                        for d in range(8):
                            yield (lambda m=m, lc=lc, d=d, c=ps_cell:
                                   mk_mm(m, lc, d, c))
                    else:
                        yield (lambda m=m, lc=lc, c=ps_cell:
                               [mk_mm(m, lc, d, c) for d in range(8)])
                yield from rope_chunk_ops(qp, lc, "q")

            def outproj_ops(lc, on):
                """Out-projection for l-chunk lc from on[pair] tiles."""
                for jb in range(2):
                    for lm in range(4):
                        def mk(jb=jb, lm=lm, lc=lc, on=on):
                            yps = auxp.tile([128, 512], F32,
                                            name=f"yps_{lc}_{lm}_{jb}",
                                            tag="aux")
                            nc.tensor.matmul(yps[:], on[1][:, ts(lm, 128)],
                                             w_o[:, 1, ts(jb, 512)],
                                             start=True, stop=False)
                            nc.tensor.matmul(yps[:], on[0][:, ts(lm, 128)],
                                             w_o[:, 0, ts(jb, 512)],
                                             start=False, stop=True)
                            ysb = ypool.tile([128, 512], DT,
                                             name=f"ysb_{lc}_{lm}_{jb}",
                                             tag="y")
                            nc.vector.tensor_copy(ysb[:], yps[:])
                            nc.sync.dma_start(
                                out=y.ap()[512 * lc + 128 * lm:
                                           512 * lc + 128 * lm + 128,
                                           ts(jb, 512)],
                                in_=ysb[:])
                        yield mk

            # ================= instruction stream =================
            # Prefix: K/V chunk 0, Q chunk 0 (with RoPE), so lc0/kb0-3
            # attention can start; everything else interleaves as filler.
            for op in kproj_chunk_ops(0):
                op()
            for op in qproj_chunk_ops(0):
                op()

            def attention_lc(lc, fillers, per_step, late_fillers=None,
                             on=None, skip_edges=True):
                """Attention for l-chunk lc.  `fillers` is an iterator of
                closures drained `per_step` per kb step (skipping the steps
                adjacent to pair boundaries so the normalize chain isn't
                queued behind filler DVE work).  `late_fillers` drains only
                during the second (pr=0) pair, after pair 1's normalize."""
                if on is None:
                    on = {}

                def drain(src, n):
                    for _ in range(n):
                        op = next(src, None)
                        if op is None:
                            return
                        op()

                for pr in (1, 0):
                    pvt = pvp.tile([128, 512], F32, name=f"pv_{lc}_{pr}",
                                   tag="pv")
                    dnt = dnp.tile([128, 512], F32, name=f"dn_{lc}_{pr}",
                                   tag="dn")
                    pend = []

                    def emit_pv(kb, ex_w, pvt=pvt, dnt=dnt, pr=pr):
                        vt = v_tiles[kb]
                        nc.tensor.matmul(pvt[0:64, :],
                                         vt[:, 128 * pr:128 * pr + 64],
                                         ex_w[:, 0:512],
                                         start=(kb == 0), stop=(kb == 15))
                        nc.tensor.matmul(pvt[64:128, :],
                                         vt[:, 128 * pr + 64:128 * pr + 128],
                                         ex_w[:, 512:1024],
                                         start=(kb == 0), stop=(kb == 15))
                        nc.tensor.matmul(dnt[0:64, :], vmr_t[:, kb, :],
                                         ex_w[:, 0:512],
                                         start=(kb == 0), stop=(kb == 15))
                        nc.tensor.matmul(dnt[64:128, :], vmr_t[:, kb, :],
                                         ex_w[:, 512:1024],
                                         start=(kb == 0), stop=(kb == 15))

                    for kb in range(16):
                        if not skip_edges or 2 <= kb < 14:
                            drain(fillers, per_step)
                            if pr == 0 and late_fillers is not None:
                                drain(late_fillers, 1)
                        wide = scwp.tile([128, 1024], F32,
                                         name=f"sc_{lc}_{pr}_{kb}", tag="sc")
                        nc.tensor.matmul(wide[:, 0:512],
                                         kp[pr][0:64, ts(kb, 128)],
                                         qp[pr][0:64, ts(lc, 512)],
                                         start=True, stop=True)
                        nc.tensor.matmul(wide[:, 512:1024],
                                         kp[pr][64:128, ts(kb, 128)],
                                         qp[pr][64:128, ts(lc, 512)],
                                         start=True, stop=True)
                        ex_w = epool.tile([128, 1024], DT,
                                          name=f"ex_{lc}_{pr}_{kb}", tag="ex")
                        nc.scalar.activation(ex_w[:], wide[:], AF.Exp,
                                             scale=SCALE)
                        pend.append((kb, ex_w))
                        if len(pend) > LAG:
                            emit_pv(*pend.pop(0))
                    while pend:
                        emit_pv(*pend.pop(0))

                    # normalize: rc = 1/denoms (already broadcast across
                    # partitions), on = pv * rc
                    rc = rcpool.tile([128, 512], F32, name=f"rc_{lc}_{pr}",
                                     tag="rc")
                    nc.vector.reciprocal_approx_fast(out=rc[:], in_=dnt[:])
                    on[pr] = onpool.tile([128, 512], DT, name=f"on_{lc}_{pr}",
                                         tag="on")
                    nc.vector.tensor_mul(on[pr][:], pvt[:], rc[:])
                return on

            # filler schedules per l-chunk
            def chain(*gens):
                for g in gens:
                    yield from g

            def eager_dma(gen):
                """Run the generator's leading DMA closure now (prefetch);
                yield the remaining compute closures lazily."""
                next(gen)()
                return gen

            def outproj_pair1_ops(on, ysb1s):
                """Pair-1 half of the lc3 out-projection: runs as late
                fillers inside lc3's pair-0 loop; partials parked in SBUF."""
                for jb in range(2):
                    for lm in range(4):
                        def mk(jb=jb, lm=lm, on=on):
                            yps = auxp.tile([128, 512], F32,
                                            name=f"y1ps_{lm}_{jb}", tag="aux")
                            nc.tensor.matmul(yps[:], on[1][:, ts(lm, 128)],
                                             w_o[:, 1, ts(jb, 512)],
                                             start=True, stop=True)
                            ysb1 = ypool.tile([128, 512], DT,
                                              name=f"ysb1_{lm}_{jb}",
                                              tag=f"y1_{lm}_{jb}")
                            nc.vector.tensor_copy(ysb1[:], yps[:])
                            ysb1s[(jb, lm)] = ysb1
                        yield mk

            def outproj_pair0_tail(on, ysb1s):
                for jb in range(2):
                    for lm in range(4):
                        yps = auxp.tile([128, 512], F32,
                                        name=f"y0ps_{lm}_{jb}", tag="aux")
                        nc.tensor.matmul(yps[:], on[0][:, ts(lm, 128)],
                                         w_o[:, 0, ts(jb, 512)],
                                         start=True, stop=True)
                        ysb = ypool.tile([128, 512], DT,
                                         name=f"ysbt_{lm}_{jb}", tag="y")
                        nc.vector.tensor_add(ysb[:], ysb1s[(jb, lm)][:],
                                             yps[:])
                        nc.sync.dma_start(
                            out=y.ap()[512 * 3 + 128 * lm:
                                       512 * 3 + 128 * lm + 128, ts(jb, 512)],
                            in_=ysb[:])

            # prefetch the remaining eT chunks and the lc1 xT chunk now so
            # the lc0 filler windows line up (chunk c complete by kb=4c):
            # exactly 8 compute closures per K chunk at 2 drains/step.
            lc0_fillers = chain(eager_dma(kproj_chunk_ops(1)),
                                eager_dma(kproj_chunk_ops(2)),
                                eager_dma(kproj_chunk_ops(3)),
                                eager_dma(qproj_chunk_ops(1)))
            on0 = attention_lc(0, lc0_fillers, per_step=2, skip_edges=False)
            on1 = attention_lc(1, chain(outproj_ops(0, on0),
                                        qproj_chunk_ops(2, fine=True)),
                               per_step=2)
            on2 = attention_lc(2, chain(outproj_ops(1, on1),
                                        qproj_chunk_ops(3, fine=True)),
                               per_step=2)
            on3 = {}
            ysb1s = {}
            attention_lc(3, outproj_ops(2, on2), per_step=1, on=on3,
                         late_fillers=outproj_pair1_ops(on3, ysb1s))
            outproj_pair0_tail(on3, ysb1s)

    nc.compile()
    return nc


def make_in_maps(x, encoder_inputs, key_padding_mask, Wq, bq, Wk, bk, Wv, bv, Wo):
    f32 = np.float32
    import ml_dtypes
    mmdt = ml_dtypes.bfloat16
    x = np.asarray(x, dtype=f32)
    enc = np.asarray(encoder_inputs, dtype=f32)
    mask = np.asarray(key_padding_mask)
    Wq = np.asarray(Wq, dtype=f32); bq = np.asarray(bq, dtype=f32)
    Wk = np.asarray(Wk, dtype=f32); bk = np.asarray(bk, dtype=f32)
    Wv = np.asarray(Wv, dtype=f32); bv = np.asarray(bv, dtype=f32)
    Wo = np.asarray(Wo, dtype=f32)

    inv_freq = (1.0 / (10000.0 ** (np.arange(0, Dh, 2, dtype=f32) / f32(Dh)))).astype(f32)
    ang = np.arange(S, dtype=f32)[:, None] * inv_freq[None, :]   # [S, 32]
    costab = np.tile(np.ascontiguousarray(np.cos(ang).T), (4, 1)).astype(f32)  # [128,S]
    sintab = np.tile(np.ascontiguousarray(np.sin(ang).T), (4, 1)).astype(f32)

    # signed swap permutation for RoPE: out = P^T x ->
    #   e-rows (32-blk 0 of each 64-blk): -x[o-row];  o-rows: +x[e-row]
    rper = np.zeros((128, 128), f32)
    for base in (0, 64):
        for i in range(32):
            rper[base + 32 + i, base + i] = -1.0
            rper[base + i, base + 32 + i] = 1.0

    xTb = [np.ascontiguousarray(x[b].T) for b in range(B)]
    eTb = [np.ascontiguousarray(enc[b].T) for b in range(B)]
    maskb = [np.ascontiguousarray(mask[b].astype(f32).reshape(16, 128).T)
             for b in range(B)]

    in_maps = []
    for core in range(NCORES):
        b = core // 4
        heads = [(core % 4) * HPC + i for i in range(HPC)]
        # interleaved per-head [e|o] ordering (32-blocks): h0e h0o h1e h1o ...
        eo = np.concatenate(
            [np.concatenate([64 * h + np.arange(0, 64, 2),
                             64 * h + np.arange(1, 64, 2)]) for h in heads])
        nat = np.concatenate([64 * h + np.arange(64) for h in heads])

        qkb = np.stack([bq[eo[:128]], bq[eo[128:]],
                        bk[eo[:128]], bk[eo[128:]]], axis=1)
        qkb = np.ascontiguousarray(qkb.astype(f32))
        # smalls: [0:256] = bv (natural head order); [320:448] = 1.0
        smalls = np.zeros((1, 512), f32)
        smalls[0, :Dc] = bv[nat]
        smalls[0, 320:448] = 1.0
        # replicated mask for denominator matmuls: [128, 16*64]
        vmaskr = np.repeat(maskb[b][:, :, None], 64, axis=2).reshape(128, 16 * 64)

        in_maps.append({
            "xT": xTb[b].astype(mmdt),
            "eT": eTb[b].astype(mmdt),
            "wq": np.ascontiguousarray(Wq[eo, :].T).astype(mmdt),
            "wk": np.ascontiguousarray(Wk[eo, :].T).astype(mmdt),
            "wv": np.ascontiguousarray(Wv[nat, :].T).astype(mmdt),
            "wo": np.ascontiguousarray(Wo[:, nat].T).astype(mmdt),
            "qkb": qkb,
            "smalls": smalls.astype(mmdt),
            "cost": costab.astype(mmdt),
            "sint": sintab.astype(mmdt),
            "rper": rper.astype(mmdt),
            "vmaskr": np.ascontiguousarray(vmaskr).astype(mmdt),
            "vmask": maskb[b],
        })
    return in_maps


_CACHE = {}


def _get_nc():
    if "nc" not in _CACHE:
        _CACHE["nc"] = build_nc()
    return _CACHE["nc"]


def kernel(x, encoder_inputs, key_padding_mask, Wq, bq, Wk, bk, Wv, bv, Wo, bo,
           _results_hook=None):
    nc = _get_nc()
    in_maps = make_in_maps(x, encoder_inputs, key_padding_mask,
                           Wq, bq, Wk, bk, Wv, bv, Wo)
    res = bass_utils.run_bass_kernel_spmd(nc, in_maps, list(range(NCORES)))
    if _results_hook is not None:
        _results_hook(res)
    bo = np.asarray(bo, dtype=np.float32)
    out = np.empty((B, L, D), np.float32)
    for b in range(B):
        acc = res.results[4 * b]["y"].astype(np.float32)
        for c in range(4 * b + 1, 4 * b + 4):
            acc = acc + res.results[c]["y"].astype(np.float32)
        out[b] = acc + bo[None, :]
    return out
